# revision 1
# baseline (speedup 1.0000x reference)
"""AttnRefine kernel for 8 Trainium2 NeuronCores.

Strategy (matches the sharding hint): pure data parallelism — the batch of 16
is sharded 2-per-core across the 8 NeuronCores; the small conv parameters are
replicated. The windowed correlation is fully local per-sample, so no
collectives are needed. The whole forward pass is compiled to the NeuronCores
through the PJRT backend and executed SPMD; outputs are gathered back to a
single full-shape result.
"""
import numpy as np
import jax
import jax.numpy as jnp
from jax.sharding import Mesh, PartitionSpec as P, NamedSharding

B, H, W = 16, 32, 64
SH = SW = 5
C = 64
N_CORES = 8


def _conv(x, w, b, stride=1):
    p = w.shape[2] // 2
    y = jax.lax.conv_general_dilated(x, w, (stride, stride), [(p, p), (p, p)],
                                     dimension_numbers=('NCHW', 'OIHW', 'NCHW'))
    return y + b[None, :, None, None]


def _lrelu(x):
    return jnp.where(x >= 0, x, 0.2 * x)


def _up2_ac(x):
    def interp(x, axis, n_in):
        n_out = 2 * n_in
        pos = jnp.linspace(0.0, n_in - 1.0, n_out)
        lo = jnp.floor(pos).astype(jnp.int32)
        hi = jnp.clip(lo + 1, 0, n_in - 1)
        frac = pos - lo.astype(pos.dtype)
        shp = [1] * x.ndim
        shp[axis] = n_out
        frac = frac.reshape(shp).astype(x.dtype)
        return jnp.take(x, lo, axis=axis) * (1 - frac) + jnp.take(x, hi, axis=axis) * frac
    x = interp(x, 2, x.shape[2])
    x = interp(x, 3, x.shape[3])
    return x


def _resblock(x, pr):
    w1, b1, w2, b2 = pr
    return _conv(_lrelu(_conv(x, w1, b1)), w2, b2) + x


def _encode(img, p):
    e1 = _lrelu(_conv(img, *p['enc1'][0]))
    e2 = _lrelu(_conv(e1, *p['enc2'][0], stride=2))
    e2 = _lrelu(_conv(e2, *p['enc2'][1]))
    e3 = _lrelu(_conv(e2, *p['enc3'][0], stride=2))
    e3 = _conv(e3, *p['enc3'][1])
    return e1, e2, e3


def _forward(coarse, neighbors, p):
    bs = coarse.shape[0]
    e1, e2, cpat = _encode(coarse, p)
    nb1 = _encode(neighbors[:, :3], p)[2]
    nb2 = _encode(neighbors[:, 3:6], p)[2]
    nbp = jnp.concatenate([nb1, nb2], axis=1)
    pad = jnp.pad(nbp, ((0, 0), (0, 0), (SH // 2, SH // 2), (SW // 2, SW // 2)),
                  constant_values=1e-6)
    win = jnp.stack([pad[:, :, dy:dy + H, dx:dx + W]
                     for dy in range(SH) for dx in range(SW)], axis=2)
    win = win.reshape(bs, 2, C, SH * SW, H, W)
    num = jnp.einsum('bnckhw,bchw->bnkhw', win, cpat)
    corr = num / jnp.sqrt((win * win).sum(axis=2))
    sm = jax.nn.softmax(corr, axis=2)
    idx = jnp.argmax(sm, axis=2)
    offset = jnp.stack([idx // SW, idx % SH], axis=2).astype(jnp.float32) - (SW // 2)
    agg = jnp.einsum('bnckhw,bnkhw->bnchw', win, sm).reshape(bs, 2 * C, H, W)
    x = jnp.concatenate([cpat, agg], axis=1)
    x = _lrelu(_conv(x, *p['dec3'][0]))
    x = _lrelu(_conv(x, *p['dec3'][1]))
    x = _resblock(x, p['dec3_rb1'])
    x = _resblock(x, p['dec3_rb2'])
    d2 = _up2_ac(x)
    d2 = _lrelu(_conv(d2, *p['dec2'][0]))
    d2 = _lrelu(_conv(d2, *p['dec2'][1]))
    d1 = _up2_ac(d2 + e2)
    d1 = _lrelu(_conv(d1, *p['dec1'][0]))
    d1 = _lrelu(_conv(d1, *p['dec1'][1]))
    out = _conv(_resblock(d1 + e1, p['out_rb']), *p['out'])
    return out, offset


_COMPILED = {}


def _get_compiled():
    if 'fn' in _COMPILED:
        return _COMPILED['fn'], _COMPILED['mesh']
    devices = jax.devices()[:N_CORES]
    mesh = Mesh(np.asarray(devices), ('b',))
    data_sh = NamedSharding(mesh, P('b'))
    repl_sh = NamedSharding(mesh, P())

    def fwd(coarse, neighbors, params):
        return _forward(coarse, neighbors, params)

    fn = jax.jit(
        fwd,
        in_shardings=(data_sh, data_sh, repl_sh),
        out_shardings=(data_sh, data_sh),
    )
    _COMPILED['fn'] = fn
    _COMPILED['mesh'] = mesh
    return fn, mesh


def kernel(coarse, neighbors, params):
    fn, mesh = _get_compiled()
    coarse = jnp.asarray(np.asarray(coarse, dtype=np.float32))
    neighbors = jnp.asarray(np.asarray(neighbors, dtype=np.float32))
    params = jax.tree.map(lambda a: jnp.asarray(np.asarray(a, dtype=np.float32)),
                          params)
    out, offset = fn(coarse, neighbors, params)
    return np.asarray(out), np.asarray(offset)


# revision 2
# speedup vs baseline: 3.6604x; 3.6604x over previous
"""AttnRefine v2: shard_map data-parallel, batched encoder, slice-based
correlation, matmul (gather-free) bilinear upsampling."""
import numpy as np
import jax
import jax.numpy as jnp
from jax.sharding import Mesh, PartitionSpec as P, NamedSharding
from jax.experimental.shard_map import shard_map

B, H, W = 16, 32, 64
SH = SW = 5
C = 64
N_CORES = 8


def _conv(x, w, b, stride=1):
    p = w.shape[2] // 2
    y = jax.lax.conv_general_dilated(x, w, (stride, stride), [(p, p), (p, p)],
                                     dimension_numbers=('NCHW', 'OIHW', 'NCHW'))
    return y + b[None, :, None, None]


def _lrelu(x):
    return jnp.where(x >= 0, x, 0.2 * x)


def _up_mat(n_in):
    # bilinear x2 align_corners=True as a dense [2n, n] fp32 matrix
    n_out = 2 * n_in
    pos = np.linspace(0.0, n_in - 1.0, n_out).astype(np.float32)
    lo = np.floor(pos).astype(np.int64)
    hi = np.clip(lo + 1, 0, n_in - 1)
    frac = (pos - lo.astype(np.float32)).astype(np.float32)
    A = np.zeros((n_out, n_in), np.float32)
    A[np.arange(n_out), lo] += (1.0 - frac)
    A[np.arange(n_out), hi] += frac
    return A


_A32 = _up_mat(32)
_A64 = _up_mat(64)
_A128 = _up_mat(128)
_UPMATS = {32: _A32, 64: _A64, 128: _A128}


def _up2_ac(x):
    Ah = jnp.asarray(_UPMATS[x.shape[2]])
    Aw = jnp.asarray(_UPMATS[x.shape[3]])
    x = jnp.einsum('Hh,bchw->bcHw', Ah, x)
    x = jnp.einsum('Ww,bchw->bchW', Aw, x)
    return x


def _resblock(x, pr):
    w1, b1, w2, b2 = pr
    return _conv(_lrelu(_conv(x, w1, b1)), w2, b2) + x


def _encode(img, p):
    e1 = _lrelu(_conv(img, *p['enc1'][0]))
    e2 = _lrelu(_conv(e1, *p['enc2'][0], stride=2))
    e2 = _lrelu(_conv(e2, *p['enc2'][1]))
    e3 = _lrelu(_conv(e2, *p['enc3'][0], stride=2))
    e3 = _conv(e3, *p['enc3'][1])
    return e1, e2, e3


def _forward(coarse, neighbors, p):
    bs = coarse.shape[0]
    # batch the three encoder passes into one
    imgs = jnp.concatenate([coarse, neighbors[:, :3], neighbors[:, 3:6]], axis=0)
    e1a, e2a, e3a = _encode(imgs, p)
    e1, e2, cpat = e1a[:bs], e2a[:bs], e3a[:bs]
    nb1, nb2 = e3a[bs:2 * bs], e3a[2 * bs:]

    nbp = jnp.concatenate([nb1, nb2], axis=1)
    pad = jnp.pad(nbp, ((0, 0), (0, 0), (2, 2), (2, 2)), constant_values=1e-6)
    pad5 = pad.reshape(bs, 2, C, H + 4, W + 4)
    cpat5 = cpat[:, None]                      # [bs,1,C,H,W]

    nums, dens = [], []
    for dy in range(SH):
        for dx in range(SW):
            wk = pad5[:, :, :, dy:dy + H, dx:dx + W]
            nums.append((wk * cpat5).sum(2))
            dens.append((wk * wk).sum(2))
    num = jnp.stack(nums, axis=2)              # [bs,2,25,H,W]
    den = jnp.stack(dens, axis=2)
    corr = num / jnp.sqrt(den)
    sm = jax.nn.softmax(corr, axis=2)
    idx = jnp.argmax(sm, axis=2)
    offset = jnp.stack([idx // SW, idx % SH], axis=2).astype(jnp.float32) - (SW // 2)

    agg = jnp.zeros((bs, 2, C, H, W), jnp.float32)
    k = 0
    for dy in range(SH):
        for dx in range(SW):
            wk = pad5[:, :, :, dy:dy + H, dx:dx + W]
            agg = agg + wk * sm[:, :, k][:, :, None]
            k += 1
    agg = agg.reshape(bs, 2 * C, H, W)

    x = jnp.concatenate([cpat, agg], axis=1)
    x = _lrelu(_conv(x, *p['dec3'][0]))
    x = _lrelu(_conv(x, *p['dec3'][1]))
    x = _resblock(x, p['dec3_rb1'])
    x = _resblock(x, p['dec3_rb2'])
    d2 = _up2_ac(x)
    d2 = _lrelu(_conv(d2, *p['dec2'][0]))
    d2 = _lrelu(_conv(d2, *p['dec2'][1]))
    d1 = _up2_ac(d2 + e2)
    d1 = _lrelu(_conv(d1, *p['dec1'][0]))
    d1 = _lrelu(_conv(d1, *p['dec1'][1]))
    out = _conv(_resblock(d1 + e1, p['out_rb']), *p['out'])
    return out, offset


_COMPILED = {}


def _get_compiled():
    if 'fn' in _COMPILED:
        return _COMPILED['fn'], _COMPILED['mesh']
    devices = jax.devices()[:N_CORES]
    mesh = Mesh(np.asarray(devices), ('b',))
    data_sh = NamedSharding(mesh, P('b'))

    fwd = shard_map(_forward, mesh=mesh,
                    in_specs=(P('b'), P('b'), P()),
                    out_specs=(P('b'), P('b')),
                    check_rep=False)
    fn = jax.jit(fwd,
                 in_shardings=(data_sh, data_sh, NamedSharding(mesh, P())),
                 out_shardings=(data_sh, data_sh))
    _COMPILED['fn'] = fn
    _COMPILED['mesh'] = mesh
    return fn, mesh


def kernel(coarse, neighbors, params):
    fn, mesh = _get_compiled()
    coarse = jnp.asarray(np.asarray(coarse, dtype=np.float32))
    neighbors = jnp.asarray(np.asarray(neighbors, dtype=np.float32))
    params = jax.tree.map(lambda a: jnp.asarray(np.asarray(a, dtype=np.float32)),
                          params)
    out, offset = fn(coarse, neighbors, params)
    return np.asarray(out), np.asarray(offset)
